# revision 94
# baseline (speedup 1.0000x reference)
"""CapsuleNetwork Trainium2 kernel (Bass/Tile), 8-core data parallel. v3.

Math (validated vs reference):
  p = x @ Wp + bp, viewed [B, n=8, d=16]; squash scale gp[b,n] = gamma(|p_n|^2)
  G[b,n,m] = u_hat_n . u_hat_m = gp_n gp_m (p_n K p_m), K = Wflat Wflat^T.
  Symmetric 5-slab Gram: 36 unique (n,m) pairs in 5 z-slabs of 128 partition
  slots; slot j=(n,d) computes z_j = sum_d' K[nd, m d'] p[m d'] so the
  elementwise factor is always p[j] itself.  e = z .* p (fp16), G entries via
  per-shift mask matmuls with stride-9 PSUM column writes.

  Routing with deferred softmax (Z cancels):
    eb = exp(b); A[n] = sum_m G[nm] eb_m; q* = sum_n eb_n A_n; Z = sum eb
    gamma(q)*a = sqrt(q*) * A / (Z^2 + q*)   [b += this]
    final: w_n = c_n gp_n gam = eb_n gp_n sqrt(q*) / (Z^2 + q*)
  v = (p .* w_bcast) @ Wflat  (gam folded into w; v fp16 out).

All PE operands fp16; xt/v DMA fp16.  PSUM: zsg tile [P,1280] (p accum and
sg region alias the z slabs), s2w [P,320] (wbt+wtT), s2v [P,512] per tile.
"""

import numpy as np

import concourse.bass as bass
import concourse.tile as tile
from concourse import mybir
from concourse.bass_utils import run_bass_kernel_spmd
from concourse.vector_clock import ScopedClock

F32 = mybir.dt.float32
F16 = mybir.dt.float16
AF = mybir.ActivationFunctionType
AX = mybir.AxisListType
ALU = mybir.AluOpType

N_CORES = 8
B_FULL, IN_DIM, OUT_DIM = 16384, 1024, 512
N_CAPS, CAP_DIM = 8, 16
ND = N_CAPS * CAP_DIM          # 128
B_CORE = B_FULL // N_CORES     # 2048
P = 128
PB = 2 * P                     # batch per pair (2 tiles)
K_CHUNKS = IN_DIM // P         # 8
N_SLABS = 5
GT = 80                        # per-tile sg stride in PSUM (64 G + 8 sq + pad)
GS = 72                        # per-tile G+sq cols kept in SBUF

# slab structure: runs of (shift, 'L'/'U') per slab; partition j=(n,d):
#   L shift t: n >= t, m = n-t ; U shift s: n <= 7-s, m = n+s
SLAB_RUNS = [
    [(0, 'L')],
    [(7, 'U'), (1, 'L')],
    [(6, 'U'), (2, 'L')],
    [(5, 'U'), (3, 'L')],
    [(4, 'L')],
]


def _runs():
    """[(slab, t, real, ns(list), mask_col_off)] with global mask col offsets."""
    out, off = [], 0
    for s, runs in enumerate(SLAB_RUNS):
        for (t, real) in runs:
            ns = list(range(t, 8)) if real == 'L' else list(range(0, 8 - t))
            out.append((s, t, real, ns, off))
            off += len(ns)
    assert off == 36
    return out


RUNS = _runs()

# cpack (fp16) column layout
C_WPC = 0
C_KT = C_WPC + IN_DIM                 # 1024
C_NMASK = C_KT + N_SLABS * ND         # 1664
C_GMASK = C_NMASK + N_CAPS            # 1672
C_BP = C_GMASK + 36                   # 1708
C_IDENT = C_BP + 1                    # 1709
C_MM2 = C_IDENT + P                   # 1837
C_WFLAT = C_MM2 + 2 * P               # 2093
CPACK = C_WFLAT + OUT_DIM             # 2605


def _patched_drain_and_barrier(self, tick_clock, wait_clock):
    # Walrus in this env allows at most ONE sem wait per instruction; the
    # stock tail drain accumulates one wait per live sem.  Collect waits on
    # a NOP, then re-emit one wait per NOP.
    nc = self.nc
    collector = nc.sync.nop()
    wait_clock.add_sem_waits(
        collector.ins, ScopedClock({None: tick_clock.global_clock})
    )
    si = collector.ins.sync_info
    waits = list(si.on_wait) if si is not None else []
    if len(waits) > 1:
        collector.ins.sync_info = mybir.SyncInfo(on_wait=waits[:1], on_update=[])
        for w in waits[1:]:
            n = nc.sync.nop()
            n.ins.sync_info = mybir.SyncInfo(on_wait=[w], on_update=[])
    nc.sync.drain()
    nc.all_engine_barrier()
    popped = nc._tile_sem_poison_stack.pop()
    assert popped is self._sem_poison
    nc.clear_and_free_semaphores(list(self.sems.allocated().values()))
    nc.all_engine_barrier()


tile.TileContext._drain_and_barrier = _patched_drain_and_barrier


def _split_multi_waits(nc):
    """Walrus here accepts at most one sem wait per instruction.  Tile's
    wait-assignment can attach several; split the extras onto single-wait
    NOPs inserted just before the instruction on the same engine."""
    k = 0
    for fn in nc.m.functions:
        for blk in fn.blocks:
            out = []
            for inst in blk.instructions:
                si = inst.sync_info
                if si is not None and len(si.on_wait) > 1:
                    waits = list(si.on_wait)
                    for w in waits[:-1]:
                        nop = mybir.InstNoOp(name=f"wsplit-{k}", ins=[], outs=[])
                        k += 1
                        nop.engine = inst.engine
                        nop.sync_info = mybir.SyncInfo(on_wait=[w], on_update=[])
                        nc.register_instruction(nop, overwrite=True)
                        out.append(nop)
                    inst.sync_info = mybir.SyncInfo(
                        on_wait=[waits[-1]], on_update=list(si.on_update)
                    )
                out.append(inst)
            blk.instructions = out


# instrumentation: instruction name -> step label (filled during build by
# snapshotting the module's instruction lists around each step)
INST_LABEL = {}
_LABEL_NC = [None]


def _patch_labeling():
    pass


def _label_steps(label, steps):
    nc = _LABEL_NC[0]
    if nc is None:
        return steps
    out = []

    def counts():
        return [len(b.instructions) for b in nc.m.functions[0].blocks]

    for i, fn in enumerate(steps):
        def wrapped(fn=fn, i=i):
            before = counts()
            fn()
            blocks = nc.m.functions[0].blocks
            for bi, b in enumerate(blocks):
                lo = before[bi] if bi < len(before) else 0
                for inst in list(b.instructions)[lo:]:
                    INST_LABEL[inst.name] = f"{label}.{i}"
        out.append(wrapped)
    return out


# ---------------- tunables ----------------
GROUPS = [[0, 1, 2, 3], [4, 5, 6, 7]]
# per-group engine for the wide routing muls ('V' = DVE, 'P' = Pool) and the
# narrow [P,Tg] ops (reduces and reciprocal are DVE-only)
CHAIN_WIDE = {0: 'V', 1: 'S'}
CHAIN_NARROW = {0: 'V', 1: 'V'}
# NOTE: Pool/GPSIMD cannot touch PSUM (BIR verifier) — only SBUF-resident
# work may go to Pool.
P2_ENG = 'P'                            # p2 = p*p engine: 'P' pool or 'A' act
FOLD_POOL = {0, 1, 2, 3, 4, 5}    # pairs whose gp-fold muls run on Pool
GP_POOL = True                          # gp-chain den/gpv muls on Pool
FOLD_SPLIT = True                      # per-pair (vs per-duo) gp+fold ops
E_DVE_SLABS = 5                         # e-mul slabs (must be 5: z is PSUM)
# (pr, ti) -> 'A'|'V' copy engine: group-0 pairs A/V, group-1 pairs V/A
V_COPY = {}
V_COPY.update({(p, 0): 'A' for p in range(4)})
V_COPY.update({(p, 1): 'V' for p in range(4)})
V_COPY.update({(p, 0): 'V' for p in range(4, 8)})
V_COPY.update({(p, 1): 'A' for p in range(4, 8)})
V_COPY_DEFAULT = ('A', 'V')
PSB_ENG = 'A'                           # p_sb copy engine: 'A' or 'V'
WT_ENG = 'V'                            # s2 wt copy engine
V_DMA_SPLIT = set()
PW_SPLIT = False
W_PAIR_SPLIT = True
EXP_SPLIT = False
GCOPY_V = set()                         # pairs whose G-copy runs on DVE
GPE_FOLLOW = False                      # gp-chain muls follow fold engine
P2_A = {}                               # pr -> True for p2 on ACT
S_W64 = 'P'                             # engine for S-mode [64]-wide chain ops                       # pairs whose v DMA splits per tile
S2SB_BUFS = 3
N_WARMUP = 5                            # PE p-state warmup matmuls
XT_BUFS = 4
XT_3WAY = True
# weave weights: in ('weave', chain, other, w) emit 1 chain step then w other
W_S1 = 2
W_S2 = 1
RW_CHAIN = 1                            # chain steps per weave round
RW_SOLO = 0                             # chain steps emitted before weaving


SCHED_MODE = 'stag'


def SCHEDULE(groups):
    """Emission schedule items:
      ('s1', pairs) / ('s2', pairs)          — emit those stages in sequence
      ('rw', gi, other_items, w)             — weave group gi's routing chain
                                               with the other items' steps
      ('r', [gi..])                          — routing chains interleaved
    """
    if len(groups) == 2:
        if SCHED_MODE == 'woven':
            return [
                ('s1', groups[0]),
                ('rw', 0, [('s1', groups[1])], W_S1),
                ('rw', 1, [('s2', groups[0])], W_S2),
                ('s2', groups[1]),
            ]
        if SCHED_MODE == 'woven2':
            return [
                ('s1', groups[0]),
                ('rw', 0, [('s1', groups[1])], W_S1),
                ('s2', groups[0]),
                ('r', [1]),
                ('s2', groups[1]),
            ]
        if SCHED_MODE == 'stag':
            return [
                ('s1', groups[0] + groups[1]),
                ('r', [0]),
                ('rw', 1, [('s2', groups[0])], W_S2),
                ('s2', groups[1]),
            ]
        return [
            ('s1', groups[0] + groups[1]),
            ('r', [0, 1]),
            ('s2', groups[0] + groups[1]),
        ]
    out = []
    for gi, gg in enumerate(groups):
        out.append(('s1', gg))
        if gi > 0:
            out.append(('s2', groups[gi - 1]))
        out.append(('r', [gi]))
    out.append(('s2', groups[-1]))
    return out


def build_nc(n_routing: int, n_tiles: int = B_CORE // P):
    assert n_tiles % 2 == 0
    INST_LABEL.clear()
    nc = bass.Bass()
    _LABEL_NC[0] = nc
    rows = n_tiles * P
    n_pairs = n_tiles // 2
    groups = [g for g in GROUPS if all(pr < n_pairs for pr in g)] or [
        list(range(n_pairs))
    ]

    n_blocks = (n_pairs + 1) // 2
    xt_ext = nc.declare_dram_parameter(
        "xt", [n_blocks * P, K_CHUNKS * 2 * PB], F16, isOutput=False
    )
    cp_ext = nc.declare_dram_parameter("cpack", [P, CPACK], F16, isOutput=False)
    v_ext = nc.declare_dram_parameter("v", [rows, OUT_DIM], F16, isOutput=True)

    with tile.TileContext(nc) as tc:
        with (
            tc.tile_pool(name="consts", bufs=1) as cpool,
            tc.tile_pool(name="persist", bufs=1) as ppool,
            tc.tile_pool(name="xin", bufs=XT_BUFS) as xpool,
            tc.tile_pool(name="s1sb", bufs=2) as s1pool,
            tc.tile_pool(name="rt", bufs=1) as rt,
            tc.tile_pool(name="s2sb", bufs=S2SB_BUFS) as s2sb,
        ):
            cp = cpool.tile([P, CPACK], F16)
            wpc = cp[:, C_WPC:C_WPC + IN_DIM]
            kt = cp[:, C_KT:C_KT + N_SLABS * ND]
            nmask = cp[:, C_NMASK:C_NMASK + N_CAPS]
            gmask = cp[:, C_GMASK:C_GMASK + 36]
            bp_sb = cp[:, C_BP:C_BP + 1]
            ident = cp[:, C_IDENT:C_IDENT + P]
            wflat = cp[:, C_WFLAT:C_WFLAT + OUT_DIM]
            eps_sb = cpool.tile([P, 1], F32)
            nc.gpsimd.memset(eps_sb[:], 1e-8)

            def eng(c):
                return {'V': nc.vector, 'P': nc.gpsimd, 'A': nc.scalar}[c]

            # -------- input DMAs (SP queue) --------
            x_pre = {}

            def load_xt_block(blk, mid=None):
                xt_sb = xpool.tile([P, K_CHUNKS, 2 * PB], F16, tag="xt")
                xin = xt_ext[blk * P:(blk + 1) * P, :].rearrange(
                    "p (k c) -> p k c", k=K_CHUNKS
                )
                if blk == 0:
                    nc.sync.dma_start(xt_sb[:, 0:2, :], xin[:, 0:2, :])
                    if mid is not None:
                        mid()
                    nc.sync.dma_start(xt_sb[:, 2:5, :], xin[:, 2:5, :])
                    nc.sync.dma_start(
                        xt_sb[:, 5:K_CHUNKS, :], xin[:, 5:K_CHUNKS, :]
                    )
                elif XT_3WAY:
                    nc.sync.dma_start(xt_sb[:, 0:3, :], xin[:, 0:3, :])
                    nc.sync.dma_start(xt_sb[:, 3:6, :], xin[:, 3:6, :])
                    nc.sync.dma_start(
                        xt_sb[:, 6:K_CHUNKS, :], xin[:, 6:K_CHUNKS, :]
                    )
                else:
                    nc.sync.dma_start(xt_sb[:, 0:4, :], xin[:, 0:4, :])
                    nc.sync.dma_start(
                        xt_sb[:, 4:K_CHUNKS, :], xin[:, 4:K_CHUNKS, :]
                    )
                return xt_sb

            # wpc + kt/masks/bp first so the p- and z-matmuls start ASAP;
            # ident/mm2/wflat (stage2-only) deferred behind xt1
            nc.sync.dma_start(cp[:, 0:C_KT], cp_ext[:, 0:C_KT])
            x_pre[0] = load_xt_block(0, mid=lambda: nc.sync.dma_start(
                cp[:, C_KT:C_IDENT], cp_ext[:, C_KT:C_IDENT]))
            x_pre[1] = load_xt_block(1)
            nc.sync.dma_start(cp[:, C_IDENT:CPACK], cp_ext[:, C_IDENT:CPACK])
            for blk in range(2, min(XT_BUFS, n_blocks)):
                x_pre[blk] = load_xt_block(blk)

            # -------- persistent per-group tensors --------
            p_all = ppool.tile([P, n_tiles * ND], F16)
            g_of, gp_of, w_of, base_of = {}, {}, {}, {}
            for gi, g in enumerate(groups):
                Tg = 2 * len(g)
                gid = g[0]
                base_of[gid] = 2 * g[0]
                g_of[gid] = ppool.tile([P, Tg * GS], F16, name=f"g{gid}")
                gp_of[gid] = ppool.tile([P, Tg * N_CAPS], F16, name=f"gp{gid}")
                w_of[gid] = ppool.tile([P, Tg * N_CAPS], F16, name=f"w{gid}")

            def gid_of(pr):
                for g in groups:
                    if pr in g:
                        return g[0]
                raise AssertionError

            xt_blocks = {}

            # ---------------- Stage 1 (per pair, as a step list) ----------
            def stage1_steps(pr, zsg_pool, aux_pool):
                t0 = 2 * pr
                gid = gid_of(pr)
                blk, half = pr // 2, pr % 2
                steps = []
                s = steps.append
                if half == 0:
                    pre = x_pre.pop(blk, None)
                    xt_blocks[blk] = pre if pre is not None else load_xt_block(blk)
                xt_sb = xt_blocks[blk]

                zsg = zsg_pool.tile([P, N_SLABS * PB], F32, tag="zsg")
                aux = aux_pool.tile([P, OUT_DIM], F32, tag="aux")
                p_ps = aux[:, 0:PB]     # own bank: zsg tile life starts at z

                def p_matmuls():
                    for k in range(K_CHUNKS):
                        nc.tensor.matmul(
                            p_ps,
                            wpc[:, k * P:(k + 1) * P],
                            xt_sb[:, k, half * PB:(half + 1) * PB],
                            start=(k == 0),
                            stop=(k == K_CHUNKS - 1),
                        )
                s(p_matmuls)

                p_sb = p_all[:, t0 * ND:(t0 + 2) * ND]
                if PSB_ENG == 'A':
                    s(lambda: nc.scalar.activation(
                        p_sb, p_ps, AF.Identity, bias=bp_sb[:, 0:1], scale=1.0))
                else:
                    s(lambda: nc.vector.tensor_scalar(
                        p_sb, p_ps, bp_sb[:, 0:1], None, op0=ALU.add))

                p2_sb = s1pool.tile([P, PB], F16, tag="p2")
                if P2_A.get(pr, P2_ENG == 'A'):
                    s(lambda: nc.scalar.activation(p2_sb[:], p_sb, AF.Square))
                else:
                    s(lambda: nc.gpsimd.tensor_mul(p2_sb[:], p_sb, p_sb))

                def z_matmuls():
                    for sl in range(N_SLABS):
                        nc.tensor.matmul(
                            zsg[:, sl * PB:(sl + 1) * PB],
                            kt[:, sl * ND:(sl + 1) * ND],
                            p_sb,
                            start=True,
                            stop=True,
                        )
                s(z_matmuls)

                e_sb = s1pool.tile([P, N_SLABS, PB], F16, tag="esb")
                zv = zsg[:, 0:N_SLABS * PB].rearrange(
                    "p (s b) -> p s b", s=N_SLABS
                )

                def p_bc(cnt):
                    return p_sb.rearrange("p (o b) -> p o b", o=1) \
                        .to_broadcast((P, cnt, PB))

                s(lambda: nc.vector.tensor_mul(
                    e_sb[:], zv, p_bc(N_SLABS)))

                # G + sq reduce matmuls, strided PSUM writes into sg region
                # (aliases z slab 0 cols, free after e-mul reads)
                def masks(ti):
                    base = ti * GT
                    for (sl, t, real, ns, moff) in RUNS:
                        cnt = len(ns)
                        lhsT = e_sb[:, sl, ti * P:(ti + 1) * P]
                        rhs = gmask[:, moff:moff + cnt]
                        if real == 'L':
                            offs = [9 * ns[0] - t, 9 * ns[0] - 8 * t]
                        else:
                            offs = [9 * ns[0] + t, 9 * ns[0] + 8 * t]
                        for oi, off in enumerate(offs):
                            if oi == 1 and t == 0:
                                continue
                            if cnt > 1:
                                out = zsg[:, base + off:base + off + 9 * cnt] \
                                    .rearrange("p (c z) -> p c z", z=9)[:, :, 0:1] \
                                    .rearrange("p c z -> p (c z)")
                            else:
                                out = zsg[:, base + off:base + off + 1]
                            nc.tensor.matmul(out, lhsT, rhs, start=True, stop=True)
                    nc.tensor.matmul(
                        zsg[:, base + 64:base + 72],
                        p2_sb[:, ti * P:(ti + 1) * P],
                        nmask[:],
                        start=True,
                        stop=True,
                    )
                s(lambda: masks(0))
                s(lambda: masks(1))

                # copy G+sq -> SBUF group tensor (fp16, one op per pair)
                goff = (t0 - base_of[gid]) * GS
                sgv = zsg[:, 0:2 * GT].rearrange(
                    "p (t x) -> p t x", t=2
                )[:, :, 0:GS]
                gdst = g_of[gid][:, goff:goff + 2 * GS].rearrange(
                    "p (t x) -> p t x", t=2
                )
                if pr in GCOPY_V:
                    s(lambda: nc.vector.tensor_copy(gdst, sgv))
                else:
                    s(lambda: nc.scalar.copy(gdst, sgv))
                return steps

            # gp + fold for a run of consecutive pairs (one op set per duo)
            def fold_steps(prs):
                pr0 = prs[0]
                nt = 2 * len(prs)          # tiles covered
                gid = gid_of(pr0)
                t0 = 2 * pr0
                goff = (t0 - base_of[gid]) * GS
                steps = []
                s = steps.append
                feng = nc.gpsimd if pr0 in FOLD_POOL else nc.vector
                gpe = (feng if GPE_FOLLOW else
                       (nc.gpsimd if GP_POOL else nc.vector))
                gv = g_of[gid][:, goff:goff + nt * GS].rearrange(
                    "p (t x) -> p t x", t=nt
                )
                sq_v = gv[:, :, 64:72]
                gp_run = gp_of[gid][:, (t0 - base_of[gid]) * 8:
                                    (t0 - base_of[gid] + nt) * 8]
                gpv = gp_run.rearrange("p (t n) -> p t n", t=nt)
                s1_ = rt.tile([P, 8 * nt], F32, tag=f"fs{pr0 % 2}")
                den = rt.tile([P, 8 * nt], F32, tag=f"fd{pr0 % 2}")
                rden = rt.tile([P, 8 * nt], F32, tag=f"fr{pr0 % 2}")
                s(lambda: nc.scalar.activation(
                    s1_[:].rearrange("p (t n) -> p t n", t=nt), sq_v,
                    AF.Sqrt, bias=eps_sb[:, 0:1]))
                # STT (TensorScalarPtr) is not a Pool-legal opcode
                s(lambda: nc.vector.scalar_tensor_tensor(
                    den[:].rearrange("p (t n) -> p t n", t=nt), sq_v, 1.0,
                    s1_[:].rearrange("p (t n) -> p t n", t=nt),
                    op0=ALU.add, op1=ALU.mult))
                s(lambda: nc.vector.reciprocal(rden[:], den[:]))
                s(lambda: gpe.tensor_mul(
                    gpv, sq_v, rden[:].rearrange("p (t n) -> p t n", t=nt)))
                g4 = gv[:, :, 0:64].rearrange("p t (n m) -> p t n m", n=8)
                gp_n = gp_run.rearrange("p (t n o) -> p t n o", t=nt, o=1) \
                    .to_broadcast((P, nt, 8, 8))
                gp_m = gp_run.rearrange("p (t o m) -> p t o m", t=nt, o=1) \
                    .to_broadcast((P, nt, 8, 8))
                s(lambda: feng.tensor_mul(g4, g4, gp_n))
                s(lambda: feng.tensor_mul(g4, g4, gp_m))
                return steps

            # ---------------- Routing (per group, as a step list) ----------
            def routing_steps(gid, Tg):
                """Return a list of closures, each emitting one instruction.
                Lets the scheduler interleave several groups' serial chains
                so the in-order engine queues pipeline across chains."""
                TN = Tg * N_CAPS
                tga = f"_{gid}"
                gi = [i for i, g in enumerate(groups) if g[0] == gid][0]
                wmode = CHAIN_WIDE.get(gi, 'V')
                W = eng('P' if wmode == 'S' else wmode)
                # engine for the [64]-wide chain ops (cg/gg/blog/badd/t1/w)
                W64 = eng(S_W64) if wmode == 'S' else W
                Nr = eng(CHAIN_NARROW.get(gi, 'V'))
                g_all = g_of[gid]
                gv = g_all[:].rearrange("p (t x) -> p t x", t=Tg)
                G4 = gv[:, :, 0:64].rearrange("p t (n m) -> p t n m", n=8)

                def t3(ap):
                    return ap.rearrange("p (t n) -> p t n", t=Tg)

                def bcast_m(src):   # [P, TN] -> bcast over n
                    return src.rearrange("p (t o m) -> p t o m", t=Tg, o=1) \
                        .to_broadcast((P, Tg, 8, 8))

                def bcast_t(src):   # [P, Tg] -> bcast over n: [P, Tg, 8]
                    return src.rearrange("p (t o) -> p t o", o=1) \
                        .to_broadcast((P, Tg, 8))

                A_ = rt.tile([P, TN], F32, tag="A" + tga)
                blog = rt.tile([P, TN], F32, tag="blog" + tga)
                qs = rt.tile([P, Tg], F32, tag="qs" + tga)
                s1_ = rt.tile([P, Tg], F32, tag="s1" + tga)
                dn = rt.tile([P, Tg], F32, tag="dn" + tga)
                rdn = rt.tile([P, Tg], F32, tag="rdn" + tga)
                sr = rt.tile([P, Tg], F32, tag="sr" + tga)
                eb = rt.tile([P, TN], F16, tag="eb" + tga)
                gcw = rt.tile([P, Tg * 64], F16, tag="gcw" + tga)
                Z = rt.tile([P, Tg], F32, tag="Z" + tga)
                Z2 = rt.tile([P, Tg], F32, tag="Z2" + tga)
                cg = rt.tile([P, TN], F32, tag="cg" + tga)
                gg = rt.tile([P, TN], F32, tag="gg" + tga)
                t1 = rt.tile([P, TN], F32, tag="t1" + tga)

                steps = []
                s = steps.append
                # ---- i = 0 (uniform c); free-axis reduces are DVE-only.
                # Split the opening reduce per tile-half so it starts as soon
                # as the first duos' folds land.
                h = Tg // 2
                s(lambda: nc.vector.reduce_sum(
                    t3(A_[:])[:, 0:h, :], G4[:, 0:h], axis=AX.X))
                s(lambda: nc.vector.reduce_sum(
                    t3(A_[:])[:, h:Tg, :], G4[:, h:Tg], axis=AX.X))
                s(lambda: nc.vector.reduce_sum(qs[:], t3(A_[:]), axis=AX.X))
                s(lambda: nc.scalar.activation(
                    s1_[:], qs[:], AF.Sqrt, bias=eps_sb[:, 0:1], scale=1.0 / 64))
                s(lambda: Nr.tensor_scalar(dn[:], qs[:], 1.0 / 64, 1.0,
                                           op0=ALU.mult, op1=ALU.add))
                s(lambda: nc.vector.reciprocal(rdn[:], dn[:]))
                s(lambda: Nr.scalar_tensor_tensor(sr[:], s1_[:], 1.0 / 8, rdn[:],
                                                  op0=ALU.mult, op1=ALU.mult))
                if n_routing == 1:
                    s(lambda: W.tensor_mul(
                        t3(w_of[gid][:]), t3(gp_of[gid][:]), bcast_t(sr[:])))
                    return steps
                s(lambda: W64.tensor_mul(t3(blog[:]), t3(A_[:]),
                                         bcast_t(sr[:])))

                wide_split = wmode == 'S'

                def bcast_m_half(src, lo, hi):
                    return src.rearrange("p (t o m) -> p t o m", t=Tg, o=1)[
                        :, lo:hi
                    ].to_broadcast((P, hi - lo, 8, 8))

                def gcw_half(lo, hi, engv):
                    engv.tensor_mul(
                        gcw[:].rearrange("p (t n m) -> p t n m", t=Tg, n=8)[
                            :, lo:hi
                        ],
                        G4[:, lo:hi], bcast_m_half(eb[:], lo, hi),
                    )

                for i in range(1, n_routing):
                    last = i == n_routing - 1
                    if wide_split and EXP_SPLIT:
                        s(lambda: nc.scalar.activation(
                            eb[:, 0:TN // 2], blog[:, 0:TN // 2], AF.Exp))
                        s(lambda: nc.scalar.activation(
                            eb[:, TN // 2:TN], blog[:, TN // 2:TN], AF.Exp))
                    else:
                        s(lambda: nc.scalar.activation(eb[:], blog[:], AF.Exp))
                    if last:
                        s(lambda: W64.tensor_mul(t1[:], eb[:], gp_of[gid][:]))
                    if wide_split:
                        s(lambda: gcw_half(0, h, nc.vector))
                        s(lambda: gcw_half(h, Tg, nc.gpsimd))
                    else:
                        s(lambda: W.tensor_mul(
                            gcw[:].rearrange("p (t n m) -> p t n m", t=Tg, n=8),
                            G4, bcast_m(eb[:])))
                    s(lambda: nc.vector.reduce_sum(
                        t3(A_[:]),
                        gcw[:].rearrange("p (t n m) -> p t n m", t=Tg, n=8),
                        axis=AX.X))
                    s(lambda: nc.vector.reduce_sum(Z[:], t3(eb[:]), axis=AX.X))
                    s(lambda: Nr.tensor_mul(Z2[:], Z[:], Z[:]))
                    s(lambda: W64.tensor_mul(cg[:], eb[:], A_[:]))
                    s(lambda: nc.vector.reduce_sum(qs[:], t3(cg[:]), axis=AX.X))
                    s(lambda: Nr.tensor_add(dn[:], Z2[:], qs[:]))
                    s(lambda: nc.vector.reciprocal(rdn[:], dn[:]))
                    s(lambda: nc.scalar.activation(
                        s1_[:], qs[:], AF.Sqrt, bias=eps_sb[:, 0:1]))
                    s(lambda: Nr.tensor_mul(sr[:], s1_[:], rdn[:]))
                    if not last:
                        s(lambda: W64.tensor_mul(t3(gg[:]), t3(A_[:]),
                                                 bcast_t(sr[:])))
                        s(lambda: W64.tensor_add(blog[:], blog[:], gg[:]))
                    elif W_PAIR_SPLIT:
                        def wmul(j):
                            W64.tensor_mul(
                                t3(w_of[gid][:])[:, 2 * j:2 * j + 2, :],
                                t3(t1[:])[:, 2 * j:2 * j + 2, :],
                                sr[:].rearrange("p (t o) -> p t o", o=1)[
                                    :, 2 * j:2 * j + 2
                                ].to_broadcast((P, 2, 8)),
                            )
                        for j in range(Tg // 2):
                            s(lambda j=j: wmul(j))
                    else:
                        s(lambda: W.tensor_mul(t3(w_of[gid][:]), t3(t1[:]),
                                               bcast_t(sr[:])))
                return steps

            # ---------------- Stage 2 (per pair, as a step list) ----------
            # PSUM layout inside one zsg-tag tile [P, 1280]:
            #   wt_ps fp16 @ f32 cols 0:64, wbt @ 1024:1280, v @ 0:1024
            def stage2_steps(pr, zsg_pool, v32_pool):
                t0 = 2 * pr
                gid = gid_of(pr)
                prl = (t0 - base_of[gid]) // 2   # pair index within group
                steps = []
                s = steps.append
                zsg = zsg_pool.tile([P, N_SLABS * PB], F32, tag="zsg")
                wt_ps = zsg[0:16, 0:64].bitcast(F16)        # [16, 128]
                wt_sb = s2sb.tile([16, P], F16, tag="wtsb")
                pw_sb = s2sb.tile([P, PB], F16, tag="pwsb")
                v_sb = s2sb.tile([P, 2, OUT_DIM], F16, tag="vsb")
                v32 = v32_pool.tile([P, OUT_DIM], F32, tag="aux")
                wbt = zsg[:, 1024:1280]

                s(lambda: nc.tensor.transpose(
                    wt_ps, w_of[gid][:, prl * 16:(prl + 1) * 16], ident[:]))
                if WT_ENG == 'V':
                    s(lambda: nc.vector.tensor_copy(wt_sb[:], wt_ps))
                else:
                    s(lambda: nc.scalar.copy(wt_sb[:], wt_ps))

                def mm2s():
                    for ti in range(2):
                        nc.tensor.matmul(
                            wbt[:, ti * P:(ti + 1) * P],
                            cp[0:16, C_MM2 + ti * P:C_MM2 + (ti + 1) * P],
                            wt_sb[:],
                            start=True,
                            stop=True,
                        )
                s(mm2s)
                if PW_SPLIT:
                    s(lambda: nc.vector.tensor_mul(
                        pw_sb[:, 0:P], wbt[:, 0:P],
                        p_all[:, t0 * ND:(t0 + 1) * ND]))
                    s(lambda: nc.vector.tensor_mul(
                        pw_sb[:, P:PB], wbt[:, P:PB],
                        p_all[:, (t0 + 1) * ND:(t0 + 2) * ND]))
                else:
                    s(lambda: nc.vector.tensor_mul(
                        pw_sb[:], wbt, p_all[:, t0 * ND:(t0 + 2) * ND]))

                def vstep(ti):
                    vt = v32[:] if ti == 0 else zsg[:, 0:OUT_DIM]
                    nc.tensor.matmul(
                        vt, pw_sb[:, ti * P:(ti + 1) * P], wflat[:],
                        start=True, stop=True,
                    )
                    ve = V_COPY.get((pr, ti), V_COPY_DEFAULT[(pr + ti) % 2])
                    if ve == 'A':
                        nc.scalar.copy(v_sb[:, ti, :], vt)
                    elif ve == 'V':
                        nc.vector.tensor_copy(v_sb[:, ti, :], vt)
                    else:
                        nc.gpsimd.tensor_copy(v_sb[:, ti, :], vt)
                s(lambda: vstep(0))
                if pr in V_DMA_SPLIT:
                    s(lambda: nc.sync.dma_start(
                        v_ext[t0 * P:(t0 + 1) * P, :], v_sb[:, 0, :]))
                    s(lambda: vstep(1))
                    s(lambda: nc.sync.dma_start(
                        v_ext[(t0 + 1) * P:(t0 + 2) * P, :], v_sb[:, 1, :]))
                else:
                    s(lambda: vstep(1))
                    s(lambda: nc.sync.dma_start(
                        v_ext[t0 * P:(t0 + 2) * P, :].rearrange(
                            "(q p) o -> p q o", p=P
                        ),
                        v_sb[:],
                    ))
                return steps

            # ---------------- Emission ----------------
            with (
                tc.tile_pool(name="zsg_ps", bufs=2, space="PSUM") as zsg_pool,
                tc.tile_pool(name="v32_ps", bufs=2, space="PSUM") as v32_pool,
            ):
                # PE p-state warmup: dummy back-to-back matmuls ramp the PE
                # clock to full speed before the first real matmul arrives.
                if N_WARMUP:
                    wsrc = cpool.tile([P, 256], F16, name="warm")
                    nc.gpsimd.memset(wsrc[:], 0.0)
                    wps = zsg_pool.tile([P, N_SLABS * PB], F32, tag="zsg")
                    for _ in range(N_WARMUP):
                        nc.tensor.matmul(
                            wps[:, 0:256], wsrc[:, 0:128], wsrc[:, 0:256],
                            start=True, stop=True,
                        )
                def weave2(lists):
                    """Round-robin interleave several step lists."""
                    out = []
                    for k in range(max(len(li) for li in lists)):
                        for li in lists:
                            if k < len(li):
                                out.append(li[k])
                    return out

                def steps_of(items):
                    """Flatten schedule items to one step list; consecutive
                    pairs are interleaved as duos to double pipeline depth.
                    s1 duos append their merged gp+fold step set."""
                    out = []
                    for item in items:
                        kind = item[0]
                        if kind not in ('s1', 's2'):
                            continue
                        fn = stage1_steps if kind == 's1' else stage2_steps
                        prs = [pr for pr in item[1] if pr < n_pairs]
                        for i in range(0, len(prs), 2):
                            duo_prs = prs[i:i + 2]
                            duo = [
                                _label_steps(
                                    f"{kind}p{pr}",
                                    fn(pr, zsg_pool, v32_pool),
                                )
                                for pr in duo_prs
                            ]
                            out.extend(weave2(duo))
                            if kind == 's1':
                                if FOLD_SPLIT:
                                    for fp in duo_prs:
                                        out.extend(_label_steps(
                                            f"fold{fp}", fold_steps([fp])))
                                else:
                                    out.extend(_label_steps(
                                        f"fold{duo_prs[0]}",
                                        fold_steps(duo_prs),
                                    ))
                    return out

                for item in SCHEDULE(groups):
                    kind = item[0]
                    if kind in ('s1', 's2'):
                        for st in steps_of([item]):
                            st()
                    elif kind == 'rw':
                        _, gi, other_items, w = item
                        if gi >= len(groups):
                            for st in steps_of(other_items):
                                st()
                            continue
                        chain = _label_steps(
                            f"r{gi}",
                            routing_steps(groups[gi][0], 2 * len(groups[gi])),
                        )
                        other = steps_of(other_items)
                        ci = oi = 0
                        for _ in range(RW_SOLO):
                            if ci < len(chain):
                                chain[ci]()
                                ci += 1
                        while ci < len(chain) or oi < len(other):
                            for _ in range(RW_CHAIN):
                                if ci < len(chain):
                                    chain[ci]()
                                    ci += 1
                            for _ in range(w):
                                if oi < len(other):
                                    other[oi]()
                                    oi += 1
                    elif kind == 'r':
                        lists = [
                            _label_steps(
                                f"r{gi}",
                                routing_steps(
                                    groups[gi][0], 2 * len(groups[gi])
                                ),
                            )
                            for gi in item[1] if gi < len(groups)
                        ]
                        for k in range(max(len(li) for li in lists)):
                            for li in lists:
                                if k < len(li):
                                    li[k]()

    _split_multi_waits(nc)
    return nc


def _host_consts(Wp, bp, W):
    Wp = np.asarray(Wp, dtype=np.float32)
    bp = np.asarray(bp, dtype=np.float32)
    W = np.asarray(W, dtype=np.float32)
    wflat = W.reshape(ND, OUT_DIM)
    K = wflat @ wflat.T
    cpack = np.zeros((P, CPACK), dtype=np.float32)
    # wpc[p, k*128+j] = Wp[k*128+p, j]
    cpack[:, C_WPC:C_WPC + IN_DIM] = (
        Wp.reshape(K_CHUNKS, P, P).transpose(1, 0, 2).reshape(P, IN_DIM)
    )
    kt = np.zeros((ND, N_SLABS * ND), dtype=np.float32)
    gmask = np.zeros((ND, 36), dtype=np.float32)
    for (s, t, real, ns, moff) in RUNS:
        for ci, n in enumerate(ns):
            m = n - t if real == 'L' else n + t
            for d in range(CAP_DIM):
                j = n * CAP_DIM + d
                gmask[j, moff + ci] = 1.0
                kt[m * CAP_DIM:(m + 1) * CAP_DIM, s * ND + j] = \
                    K[n * CAP_DIM + d, m * CAP_DIM:(m + 1) * CAP_DIM]
    cpack[:, C_KT:C_KT + N_SLABS * ND] = kt
    nmask = np.zeros((ND, N_CAPS), dtype=np.float32)
    for n in range(N_CAPS):
        nmask[n * CAP_DIM:(n + 1) * CAP_DIM, n] = 1.0
    cpack[:, C_NMASK:C_NMASK + N_CAPS] = nmask
    cpack[:, C_GMASK:C_GMASK + 36] = gmask
    cpack[:, C_BP] = bp.reshape(ND)
    cpack[:, C_IDENT:C_IDENT + P] = np.eye(P, dtype=np.float32)
    mm2 = np.zeros((P, 2 * P), dtype=np.float32)
    for ti in range(2):
        for n in range(N_CAPS):
            for d in range(CAP_DIM):
                mm2[ti * 8 + n, ti * P + n * CAP_DIM + d] = 1.0
    cpack[:, C_MM2:C_MM2 + 2 * P] = mm2
    cpack[:, C_WFLAT:C_WFLAT + OUT_DIM] = wflat
    return {"cpack": cpack.astype(np.float16)}


_NC_CACHE = {}
TRACE = False
LAST_RESULT = None


def make_xt(x_part):
    """[rows, 1024] f32 -> fp16 pre-transposed 2-pair-block layout:
    out[blk*128+p, k*512 + pp*256 + b] = x_part[(2*blk+pp)*256+b, k*128+p]."""
    rows = x_part.shape[0]
    n_pairs = rows // PB
    n_blocks = (n_pairs + 1) // 2
    xv = x_part.reshape(n_pairs, PB, K_CHUNKS, P)
    if n_pairs % 2:
        xv = np.concatenate([xv, np.zeros_like(xv[:1])], axis=0)
    t = xv.reshape(n_blocks, 2, PB, K_CHUNKS, P).transpose(0, 4, 3, 1, 2)
    return np.ascontiguousarray(
        t.reshape(n_blocks * P, K_CHUNKS * 2 * PB).astype(np.float16)
    )


def kernel(x, Wp, bp, W, n_routing):
    n_routing = int(n_routing)
    x = np.ascontiguousarray(np.asarray(x, dtype=np.float32))
    assert x.shape == (B_FULL, IN_DIM)

    key = (n_routing,)
    if key not in _NC_CACHE:
        _NC_CACHE[key] = build_nc(n_routing)
    nc = _NC_CACHE[key]

    consts = _host_consts(Wp, bp, W)
    in_maps = []
    for c in range(N_CORES):
        m = {"xt": make_xt(x[c * B_CORE:(c + 1) * B_CORE, :])}
        m.update(consts)
        in_maps.append(m)

    global LAST_RESULT
    res = run_bass_kernel_spmd(nc, in_maps, list(range(N_CORES)), trace=TRACE)
    LAST_RESULT = res
    out = np.concatenate([res.results[c]["v"] for c in range(N_CORES)], axis=0)
    return out.astype(np.float32)
